# revision 20
# baseline (speedup 1.0000x reference)
"""Trainium2 Bass kernel for CoherenceGate.

Math (B=8192, K=1024, T=1.0):
    u_re = psi_re @ W^T ; u_im = psi_im @ W^T
    out  = sigmoid(psi_re*u_re + psi_im*u_im + bias)

Strategy: data-parallel over B across 8 NeuronCores (1024 rows each),
W/bias replicated. On-device everything is computed in the TRANSPOSED
orientation (out^T[k, b]) so every DMA is a natural row-major transfer:

    host pre-transposes psi slices -> psiT (K x B_loc) and W -> WT (K x K)
    u^T[k,b] = sum_j WT[j,k] * psiT[j,b]      (TensorE, fp32r, j on partitions)
    out^T    = sigmoid(psiT_re .* u_re^T + psiT_im .* u_im^T + bias[k])
    host transposes the per-core out^T back.

The psiT tiles in SBUF serve double duty: matmul moving operand (rows=j)
and elementwise epilogue operand (rows=k).

Schedule: two re-phases then two im-phases, each holding all 8 PSUM banks
(4 kt x 2 b-chunks). Input DMAs are split into 128KB chunks emitted in
consumption order (psiT_re+WT first-half interleaved, WT second half, then
psiT_im) so TensorE starts within a few us of launch instead of after the
full 12MB load.
"""

import numpy as np

import concourse.bass as bass
import concourse.mybir as mybir
import concourse.tile as tile
from concourse import bacc
from concourse.bass_utils import run_bass_kernel_spmd

NC_COUNT = 8
B = 8192
K = 1024
BL = B // NC_COUNT  # 1024 rows per core

P = 128            # SBUF partitions
JT = K // P        # 8 contraction tiles
KT = K // P        # 8 output-row tiles
NB = 512           # moving free dim per matmul (fp32 max)
BC = BL // NB      # 2 b-chunks
WAVE_KT = 4        # kt tiles per wave (4 kt x 2 bc = 8 PSUM banks)

TRACE = False      # test.py sets True to collect an NTFF profile
_CACHE = {}


def _emit(tc, nc, psiT_re_d, psiT_im_d, WT_d, bias_d, outT_d):
    f32 = mybir.dt.float32
    f32r = mybir.dt.float32r

    with (
        tc.tile_pool(name="resident", bufs=1) as res,
        tc.tile_pool(name="tre", bufs=1) as tre,
        tc.tile_pool(name="psum", bufs=1, space="PSUM") as psum,
        tc.tile_pool(name="tmp", bufs=6) as tmp,
    ):
        psiT_re_sb = [
            res.tile([P, BL], f32r, tag=f"psiT_re_{jt}", name=f"psiT_re_{jt}")
            for jt in range(JT)
        ]
        psiT_im_sb = [
            res.tile([P, BL], f32r, tag=f"psiT_im_{jt}", name=f"psiT_im_{jt}")
            for jt in range(JT)
        ]
        WT_sb = [
            res.tile([P, K], f32r, tag=f"WT_{jt}", name=f"WT_{jt}")
            for jt in range(JT)
        ]
        bias_sb = res.tile([P, KT], f32, tag="bias", name="bias_sb")

        def load(sb, dram, jt, c0, c1):
            nc.sync.dma_start(
                out=sb[:, c0:c1], in_=dram[jt * P:(jt + 1) * P, c0:c1]
            )

        # ---- loads, in consumption order (HWDGE queue drains FIFO) ----
        # R1 needs psiT_re + WT[:, :512]; R2 the rest of WT; I* need psiT_im.
        # jt=0 is split finer so the first matmuls' completion sems fire as
        # early as possible during the DMA ramp.
        nc.sync.dma_start(out=bias_sb, in_=bias_d)
        load(psiT_re_sb[0], psiT_re_d, 0, 0, NB)
        load(WT_sb[0], WT_d, 0, 0, P)
        load(psiT_re_sb[0], psiT_re_d, 0, NB, BL)
        load(WT_sb[0], WT_d, 0, P, K // 2)
        for jt in range(1, JT):
            load(psiT_re_sb[jt], psiT_re_d, jt, 0, BL)
            load(WT_sb[jt], WT_d, jt, 0, K // 2)
        for jt in range(JT):
            load(WT_sb[jt], WT_d, jt, K // 2, K)
        for jt in range(JT):
            load(psiT_im_sb[jt], psiT_im_d, jt, 0, BL)

        t_re_sb = {}
        u_tiles = {}

        def open_psum(kt, bc, which):
            u_tiles[(kt, bc)] = psum.tile(
                [P, NB], f32, tag=f"u_{kt % WAVE_KT}_{bc}",
                name=f"u_{which}_{kt}_{bc}",
            )

        def mm(kt, bc, jt, psi_sb):
            nc.tensor.matmul(
                u_tiles[(kt, bc)],
                WT_sb[jt][:, kt * P:(kt + 1) * P],
                psi_sb[jt][:, bc * NB:(bc + 1) * NB],
                start=(jt == 0),
                stop=(jt == JT - 1),
            )

        def mm_wave(kts, psi_sb, which):
            # 8 open psum accumulations (4 kt x 2 bc), jt-major so each
            # matmul depends only on the jt-chunk already DMA'd.
            for kt in kts:
                for bc in range(BC):
                    open_psum(kt, bc, which)
            for jt in range(JT):
                for kt in kts:
                    for bc in range(BC):
                        mm(kt, bc, jt, psi_sb)

        def drain_re(kt, bc):
            t = tre.tile([P, NB], f32, tag=f"t_re_{kt}_{bc}",
                         name=f"t_re_{kt}_{bc}")
            t_re_sb[(kt, bc)] = t
            bsl = slice(bc * NB, (bc + 1) * NB)
            nc.vector.tensor_mul(
                t, u_tiles[(kt, bc)], psiT_re_sb[kt][:, bsl].bitcast(f32)
            )

        def drain_im(kt, bc, halves=1):
            # halves=2 shortens the trailing dependency chain after the very
            # last matmuls by pipelining half-width epilogue ops.
            w = NB // halves
            t_im = tmp.tile([P, NB], f32, tag="t_im", name=f"t_im_{kt}_{bc}")
            s = tmp.tile([P, NB], f32, tag="s", name=f"s_{kt}_{bc}")
            o = tmp.tile([P, NB], f32, tag="o", name=f"o_{kt}_{bc}")
            for h in range(halves):
                hs = slice(h * w, (h + 1) * w)
                bsl = slice(bc * NB + h * w, bc * NB + (h + 1) * w)
                nc.vector.tensor_mul(
                    t_im[:, hs], u_tiles[(kt, bc)][:, hs],
                    psiT_im_sb[kt][:, bsl].bitcast(f32),
                )
                nc.vector.tensor_add(s[:, hs], t_re_sb[(kt, bc)][:, hs], t_im[:, hs])
                nc.scalar.activation(
                    o[:, hs], s[:, hs], mybir.ActivationFunctionType.Sigmoid,
                    bias=bias_sb[:, kt:kt + 1],
                )
                nc.sync.dma_start(out=outT_d[kt * P:(kt + 1) * P, bsl], in_=o[:, hs])

        # ---- re phases: drain psum into t_re = u_re .* psiT_re ----
        for kts in (range(0, WAVE_KT), range(WAVE_KT, KT)):
            mm_wave(kts, psiT_re_sb, "re")
            for kt in kts:
                for bc in range(BC):
                    drain_re(kt, bc)

        # ---- im phase 1: jt-major (psiT_im still arriving) ----
        mm_wave(range(0, WAVE_KT), psiT_im_sb, "im")
        for kt in range(0, WAVE_KT):
            for bc in range(BC):
                drain_im(kt, bc)

        # ---- im phase 2: all data resident; contiguous per-psum groups so
        # each epilogue overlaps the next group's matmuls (short tail) ----
        for kt in range(WAVE_KT, KT):
            for bc in range(BC):
                open_psum(kt, bc, "im")
                for jt in range(JT):
                    mm(kt, bc, jt, psiT_im_sb)
                last = kt == KT - 1 and bc == BC - 1
                drain_im(kt, bc, halves=2 if last else 1)


def _build():
    if "nc" in _CACHE:
        return _CACHE["nc"]
    nc = bacc.Bacc("TRN2", target_bir_lowering=False, debug=False)
    f32 = mybir.dt.float32
    f32r = mybir.dt.float32r
    psiT_re_d = nc.dram_tensor("psiT_re", [K, BL], f32r, kind="ExternalInput").ap()
    psiT_im_d = nc.dram_tensor("psiT_im", [K, BL], f32r, kind="ExternalInput").ap()
    WT_d = nc.dram_tensor("WT", [K, K], f32r, kind="ExternalInput").ap()
    bias_d = nc.dram_tensor("bias", [P, KT], f32, kind="ExternalInput").ap()
    outT_d = nc.dram_tensor("outT", [K, BL], f32, kind="ExternalOutput").ap()
    with tile.TileContext(nc) as tc:
        _emit(tc, nc, psiT_re_d, psiT_im_d, WT_d, bias_d, outT_d)
    nc.compile()
    _CACHE["nc"] = nc
    return nc


def make_in_maps(psi_re, psi_im, W, bias):
    WT = np.ascontiguousarray(np.asarray(W, dtype=np.float32).T)
    bias = np.asarray(bias, dtype=np.float32)
    # bias_sb[p, t] = bias[t*128 + p]
    bias_t = np.ascontiguousarray(bias.reshape(KT, P).T)
    psi_re = np.asarray(psi_re, dtype=np.float32)
    psi_im = np.asarray(psi_im, dtype=np.float32)
    in_maps = []
    for c in range(NC_COUNT):
        sl = slice(c * BL, (c + 1) * BL)
        in_maps.append({
            "psiT_re": np.ascontiguousarray(psi_re[sl].T),
            "psiT_im": np.ascontiguousarray(psi_im[sl].T),
            "WT": WT,
            "bias": bias_t,
        })
    return in_maps


def kernel(psi_re, psi_im, W, bias):
    nc = _build()
    in_maps = make_in_maps(psi_re, psi_im, W, bias)
    r = run_bass_kernel_spmd(nc, in_maps, list(range(NC_COUNT)), trace=TRACE)
    _CACHE["last_result"] = r
    out = np.empty((B, K), dtype=np.float32)
    for c in range(NC_COUNT):
        out[c * BL:(c + 1) * BL, :] = r.results[c]["outT"].T
    return out


# revision 21
# speedup vs baseline: 1.1054x; 1.1054x over previous
"""Trainium2 Bass kernel for CoherenceGate.

Math (B=8192, K=1024, T=1.0):
    u_re = psi_re @ W^T ; u_im = psi_im @ W^T
    out  = sigmoid(psi_re*u_re + psi_im*u_im + bias)

Strategy: data-parallel over B across 8 NeuronCores (1024 rows each),
W/bias replicated. On-device everything is computed in the TRANSPOSED
orientation (out^T[k, b]) so every DMA is a natural row-major transfer:

    host pre-transposes psi slices -> psiT (K x B_loc) and W -> WT (K x K)
    u^T[k,b] = sum_j WT[j,k] * psiT[j,b]      (TensorE, fp32r, j on partitions)
    out^T    = sigmoid(psiT_re .* u_re^T + psiT_im .* u_im^T + bias[k])
    host transposes the per-core out^T back.

The psiT tiles in SBUF serve double duty: matmul moving operand (rows=j)
and elementwise epilogue operand (rows=k).

Schedule: two re-phases then two im-phases, each holding all 8 PSUM banks
(4 kt x 2 b-chunks). Input DMAs are split into 128KB chunks emitted in
consumption order (psiT_re+WT first-half interleaved, WT second half, then
psiT_im) so TensorE starts within a few us of launch instead of after the
full 12MB load.
"""

import numpy as np

import concourse.bass as bass
import concourse.mybir as mybir
import concourse.tile as tile
from concourse import bacc
from concourse.bass_utils import run_bass_kernel_spmd

NC_COUNT = 8
B = 8192
K = 1024
BL = B // NC_COUNT  # 1024 rows per core

P = 128            # SBUF partitions
JT = K // P        # 8 contraction tiles
KT = K // P        # 8 output-row tiles
NB = 512           # moving free dim per matmul (fp32 max)
BC = BL // NB      # 2 b-chunks
WAVE_KT = 4        # kt tiles per wave (4 kt x 2 bc = 8 PSUM banks)
MM_DTYPE = "bf16"  # "bf16" (half the input DMA, ~5e-4 err) or "f32r" (~2e-4 err)

TRACE = False      # test.py sets True to collect an NTFF profile
_CACHE = {}


def _emit(tc, nc, psiT_re_d, psiT_im_d, WT_d, bias_d, outT_d):
    f32 = mybir.dt.float32
    mdt = mybir.dt.bfloat16 if MM_DTYPE == "bf16" else mybir.dt.float32r

    def epi_view(ap):
        # epilogue (DVE) view of a psi tile: bf16 reads directly (mixed-
        # dtype tensor_tensor), f32r must be bitcast to plain f32
        return ap if MM_DTYPE == "bf16" else ap.bitcast(f32)

    with (
        tc.tile_pool(name="resident", bufs=1) as res,
        tc.tile_pool(name="tre", bufs=1) as tre,
        tc.tile_pool(name="psum", bufs=1, space="PSUM") as psum,
        tc.tile_pool(name="tmp", bufs=6) as tmp,
    ):
        psiT_re_sb = [
            res.tile([P, BL], mdt, tag=f"psiT_re_{jt}", name=f"psiT_re_{jt}")
            for jt in range(JT)
        ]
        psiT_im_sb = [
            res.tile([P, BL], mdt, tag=f"psiT_im_{jt}", name=f"psiT_im_{jt}")
            for jt in range(JT)
        ]
        WT_sb = [
            res.tile([P, K], mdt, tag=f"WT_{jt}", name=f"WT_{jt}")
            for jt in range(JT)
        ]
        bias_sb = res.tile([P, KT], f32, tag="bias", name="bias_sb")

        def load(sb, dram, jt, c0, c1):
            nc.sync.dma_start(
                out=sb[:, c0:c1], in_=dram[jt * P:(jt + 1) * P, c0:c1]
            )

        # ---- loads, in consumption order (HWDGE queue drains FIFO) ----
        # R1 needs psiT_re + WT[:, :512]; R2 the rest of WT; I* need psiT_im.
        # jt=0 is split finer so the first matmuls' completion sems fire as
        # early as possible during the DMA ramp.
        nc.sync.dma_start(out=bias_sb, in_=bias_d)
        load(psiT_re_sb[0], psiT_re_d, 0, 0, NB)
        load(WT_sb[0], WT_d, 0, 0, P)
        load(psiT_re_sb[0], psiT_re_d, 0, NB, BL)
        load(WT_sb[0], WT_d, 0, P, K // 2)
        for jt in range(1, JT):
            load(psiT_re_sb[jt], psiT_re_d, jt, 0, BL)
            load(WT_sb[jt], WT_d, jt, 0, K // 2)
        for jt in range(JT):
            load(WT_sb[jt], WT_d, jt, K // 2, K)
        for jt in range(JT):
            load(psiT_im_sb[jt], psiT_im_d, jt, 0, BL)

        t_re_sb = {}
        u_tiles = {}

        def open_psum(kt, bc, which):
            u_tiles[(kt, bc)] = psum.tile(
                [P, NB], f32, tag=f"u_{kt % WAVE_KT}_{bc}",
                name=f"u_{which}_{kt}_{bc}",
            )

        def mm(kt, bc, jt, psi_sb):
            nc.tensor.matmul(
                u_tiles[(kt, bc)],
                WT_sb[jt][:, kt * P:(kt + 1) * P],
                psi_sb[jt][:, bc * NB:(bc + 1) * NB],
                start=(jt == 0),
                stop=(jt == JT - 1),
            )

        def mm_wave(kts, psi_sb, which):
            # 8 open psum accumulations (4 kt x 2 bc), jt-major so each
            # matmul depends only on the jt-chunk already DMA'd.
            for kt in kts:
                for bc in range(BC):
                    open_psum(kt, bc, which)
            for jt in range(JT):
                for kt in kts:
                    for bc in range(BC):
                        mm(kt, bc, jt, psi_sb)

        def drain_re(kt, bc):
            t = tre.tile([P, NB], f32, tag=f"t_re_{kt}_{bc}",
                         name=f"t_re_{kt}_{bc}")
            t_re_sb[(kt, bc)] = t
            bsl = slice(bc * NB, (bc + 1) * NB)
            nc.vector.tensor_mul(
                t, u_tiles[(kt, bc)], epi_view(psiT_re_sb[kt][:, bsl])
            )

        def drain_im(kt, bc, halves=1):
            # halves=2 shortens the trailing dependency chain after the very
            # last matmuls by pipelining half-width epilogue ops.
            w = NB // halves
            t_im = tmp.tile([P, NB], f32, tag="t_im", name=f"t_im_{kt}_{bc}")
            s = tmp.tile([P, NB], f32, tag="s", name=f"s_{kt}_{bc}")
            o = tmp.tile([P, NB], f32, tag="o", name=f"o_{kt}_{bc}")
            for h in range(halves):
                hs = slice(h * w, (h + 1) * w)
                bsl = slice(bc * NB + h * w, bc * NB + (h + 1) * w)
                nc.vector.tensor_mul(
                    t_im[:, hs], u_tiles[(kt, bc)][:, hs],
                    epi_view(psiT_im_sb[kt][:, bsl]),
                )
                nc.vector.tensor_add(s[:, hs], t_re_sb[(kt, bc)][:, hs], t_im[:, hs])
                nc.scalar.activation(
                    o[:, hs], s[:, hs], mybir.ActivationFunctionType.Sigmoid,
                    bias=bias_sb[:, kt:kt + 1],
                )
                nc.sync.dma_start(out=outT_d[kt * P:(kt + 1) * P, bsl], in_=o[:, hs])

        # ---- re phases: drain psum into t_re = u_re .* psiT_re ----
        for kts in (range(0, WAVE_KT), range(WAVE_KT, KT)):
            mm_wave(kts, psiT_re_sb, "re")
            for kt in kts:
                for bc in range(BC):
                    drain_re(kt, bc)

        # ---- im phase 1: jt-major (psiT_im still arriving) ----
        mm_wave(range(0, WAVE_KT), psiT_im_sb, "im")
        for kt in range(0, WAVE_KT):
            for bc in range(BC):
                drain_im(kt, bc)

        # ---- im phase 2: all data resident; contiguous per-psum groups so
        # each epilogue overlaps the next group's matmuls (short tail) ----
        for kt in range(WAVE_KT, KT):
            for bc in range(BC):
                open_psum(kt, bc, "im")
                for jt in range(JT):
                    mm(kt, bc, jt, psiT_im_sb)
                last = kt == KT - 1 and bc == BC - 1
                drain_im(kt, bc, halves=2 if last else 1)


def _build():
    if "nc" in _CACHE:
        return _CACHE["nc"]
    nc = bacc.Bacc("TRN2", target_bir_lowering=False, debug=False)
    f32 = mybir.dt.float32
    mdt = mybir.dt.bfloat16 if MM_DTYPE == "bf16" else mybir.dt.float32r
    psiT_re_d = nc.dram_tensor("psiT_re", [K, BL], mdt, kind="ExternalInput").ap()
    psiT_im_d = nc.dram_tensor("psiT_im", [K, BL], mdt, kind="ExternalInput").ap()
    WT_d = nc.dram_tensor("WT", [K, K], mdt, kind="ExternalInput").ap()
    bias_d = nc.dram_tensor("bias", [P, KT], f32, kind="ExternalInput").ap()
    outT_d = nc.dram_tensor("outT", [K, BL], f32, kind="ExternalOutput").ap()
    with tile.TileContext(nc) as tc:
        _emit(tc, nc, psiT_re_d, psiT_im_d, WT_d, bias_d, outT_d)
    nc.compile()
    _CACHE["nc"] = nc
    return nc


def make_in_maps(psi_re, psi_im, W, bias):
    if MM_DTYPE == "bf16":
        import ml_dtypes
        mdt = ml_dtypes.bfloat16
    else:
        mdt = np.float32
    WT = np.ascontiguousarray(np.asarray(W, dtype=np.float32).T.astype(mdt))
    bias = np.asarray(bias, dtype=np.float32)
    # bias_sb[p, t] = bias[t*128 + p]
    bias_t = np.ascontiguousarray(bias.reshape(KT, P).T)
    psi_re = np.asarray(psi_re, dtype=np.float32)
    psi_im = np.asarray(psi_im, dtype=np.float32)
    in_maps = []
    for c in range(NC_COUNT):
        sl = slice(c * BL, (c + 1) * BL)
        in_maps.append({
            "psiT_re": np.ascontiguousarray(psi_re[sl].T.astype(mdt)),
            "psiT_im": np.ascontiguousarray(psi_im[sl].T.astype(mdt)),
            "WT": WT,
            "bias": bias_t,
        })
    return in_maps


def kernel(psi_re, psi_im, W, bias):
    nc = _build()
    in_maps = make_in_maps(psi_re, psi_im, W, bias)
    r = run_bass_kernel_spmd(nc, in_maps, list(range(NC_COUNT)), trace=TRACE)
    _CACHE["last_result"] = r
    out = np.empty((B, K), dtype=np.float32)
    for c in range(NC_COUNT):
        out[c * BL:(c + 1) * BL, :] = r.results[c]["outT"].T
    return out
